# revision 6
# baseline (speedup 1.0000x reference)
"""Trainium2 Bass kernel for nn_CustomLoss_54400055771232.

Computes, over full inputs:
    mse   = mean_c (preds - targets)^2                      # [B, T]
    w     = nee_qc * igbp_table[igbp] * koppen_table[koppen]
    bal   = (preds[..2] + preds[..0] - preds[..1])^2        # [B, T]
    out   = mean_bt(mse * w + ALPHA * bal)                  # scalar

Strategy: pure data-parallel over B across 8 NeuronCores. Per core the
[B/8, T] problem is laid out partition-chunked: partition p owns a
contiguous run of bt indices (5840 each). The igbp table gather is done
as 16 per-class masked reductions fused into single scalar_tensor_tensor
passes ((igbp==k)*y with accum_out), in bf16 for the 2x DVE mode; the
koppen gather (5 classes) materializes w2 via fused tensor_scalar
is_equal*table masks + adds in f32. Table values enter as per-partition
scalar APs (DMA'd), so the compiled NEFF is reused across calls.
Per-class/per-tile partial sums are combined on the host (f64) with the
igbp table weights; the final mean is exact linear post-processing.
"""

import sys

if "/opt/trn_rl_repo" not in sys.path:
    sys.path.insert(0, "/opt/trn_rl_repo")

import numpy as np

import concourse.bass as bass
import concourse.bacc as bacc
import concourse.tile as tile
from concourse import mybir
from concourse.bass_utils import run_bass_kernel_spmd

# Problem constants (hardcoded per harness contract).
B, T, C = 16384, 365, 6
N_IGBP, N_KOPPEN = 16, 5
ALPHA = 0.1
N_CORES = 8

B_CORE = B // N_CORES            # 2048
BT = B_CORE * T                  # 747520
P = 128                          # partitions
FP = BT // P                     # 5840 free elems per partition (16*365)
FT = 730                         # bt elems per tile per partition
NTILES = FP // FT                # 8
assert FT * NTILES == FP

f32 = mybir.dt.float32
bf16 = mybir.dt.bfloat16
i32 = mybir.dt.int32

# scalar_tensor_tensor is DVE-only (walrus rejects it on Pool), so all 16
# igbp-class masked reductions run on DVE; GPSIMD gets the reduce-over-C
# and the balance-term adds instead.
N_GPS_CLASSES = 0
# Which engine handles the reduce-over-C=6 (GPSIMD frees DVE).
AF = mybir.ActivationFunctionType
OP = mybir.AluOpType

_CACHE = {}


def _build():
    nc = bacc.Bacc("TRN2", target_bir_lowering=False, debug=False,
                   num_devices=N_CORES)

    preds = nc.dram_tensor("preds", [P, FP * C], f32, kind="ExternalInput").ap()
    targs = nc.dram_tensor("targs", [P, FP * C], f32, kind="ExternalInput").ap()
    qc = nc.dram_tensor("qc", [P, FP], f32, kind="ExternalInput").ap()
    igbp = nc.dram_tensor("igbp", [P, FP], i32, kind="ExternalInput").ap()
    kopp = nc.dram_tensor("kopp", [P, FP], i32, kind="ExternalInput").ap()
    # koppen table replicated per partition: [P, N_KOPPEN]
    ktab = nc.dram_tensor("ktab", [P, N_KOPPEN], f32, kind="ExternalInput").ap()
    # outputs: per-(tile, igbp-class) mse partial sums + per-tile balance sums
    acc_out = nc.dram_tensor("acc", [P, NTILES * N_IGBP], f32,
                             kind="ExternalOutput").ap()
    bal_out = nc.dram_tensor("bal", [P, NTILES], f32, kind="ExternalOutput").ap()

    preds3 = preds.rearrange("p (t f) -> p t f", t=NTILES)   # f = FT*C
    targs3 = targs.rearrange("p (t f) -> p t f", t=NTILES)
    qc3 = qc.rearrange("p (t f) -> p t f", t=NTILES)         # f = FT
    igbp3 = igbp.rearrange("p (t f) -> p t f", t=NTILES)
    kopp3 = kopp.rearrange("p (t f) -> p t f", t=NTILES)

    with tile.TileContext(nc) as tc:
        with (
            tc.tile_pool(name="big", bufs=2) as big,       # DMA-facing big tiles
            tc.tile_pool(name="small", bufs=2) as small,   # DMA-facing small tiles
            tc.tile_pool(name="work", bufs=2) as work,     # intermediates
            tc.tile_pool(name="accs", bufs=1) as accs,     # persistent accumulators
        ):
            ktab_t = accs.tile([P, N_KOPPEN], f32)
            nc.sync.dma_start(ktab_t[:], ktab[:])
            acc_t = accs.tile([P, NTILES * N_IGBP], f32)
            bal_t = accs.tile([P, NTILES], f32)

            for t in range(NTILES):
                p_t = big.tile([P, FT * C], f32, tag="p")
                nc.sync.dma_start(p_t[:], preds3[:, t, :])
                g_t = big.tile([P, FT * C], f32, tag="tg")
                nc.sync.dma_start(g_t[:], targs3[:, t, :])
                q_t = small.tile([P, FT], f32, tag="q")
                nc.sync.dma_start(q_t[:], qc3[:, t, :])
                ig_t = small.tile([P, FT], i32, tag="ig")
                nc.sync.dma_start(ig_t[:], igbp3[:, t, :])
                kp_t = small.tile([P, FT], i32, tag="kp")
                nc.sync.dma_start(kp_t[:], kopp3[:, t, :])

                # d = p - t  (DVE, f32 1x)
                d_t = big.tile([P, FT * C], f32, tag="d")
                nc.vector.tensor_sub(d_t[:], p_t[:], g_t[:])
                # g = d^2 (ScalarE); overwrite the targets tile (free slot)
                nc.scalar.activation(g_t[:], d_t[:], AF.Square)

                # s = sum over C=6 (GPSIMD strided adds)
                g3 = g_t[:].rearrange("p (f c) -> p f c", c=C)
                r_t = work.tile([P, FT, 3], f32, tag="r")
                nc.gpsimd.tensor_add(r_t[:], g3[:, :, 0:3], g3[:, :, 3:6])
                s_t = work.tile([P, FT], f32, tag="s")
                nc.gpsimd.tensor_add(s_t[:], r_t[:, :, 0], r_t[:, :, 1])
                nc.gpsimd.tensor_add(s_t[:], s_t[:], r_t[:, :, 2])

                # z = s * q (DVE)
                z_t = work.tile([P, FT], f32, tag="z")
                nc.vector.tensor_mul(z_t[:], s_t[:], q_t[:])

                # w2 = koppen_table[koppen] via 5-class one-hot (f32)
                m0 = work.tile([P, FT], f32, tag="m0")
                nc.vector.tensor_scalar(m0[:], kp_t[:], 0.0,
                                        ktab_t[:, 0:1], OP.is_equal, OP.mult)
                m1 = work.tile([P, FT], f32, tag="m1")
                nc.vector.tensor_scalar(m1[:], kp_t[:], 1.0,
                                        ktab_t[:, 1:2], OP.is_equal, OP.mult)
                nc.vector.tensor_add(m0[:], m0[:], m1[:])
                nc.vector.tensor_scalar(m1[:], kp_t[:], 2.0,
                                        ktab_t[:, 2:3], OP.is_equal, OP.mult)
                m2 = work.tile([P, FT], f32, tag="m2")
                nc.vector.tensor_scalar(m2[:], kp_t[:], 3.0,
                                        ktab_t[:, 3:4], OP.is_equal, OP.mult)
                nc.vector.tensor_add(m1[:], m1[:], m2[:])
                nc.vector.tensor_scalar(m2[:], kp_t[:], 4.0,
                                        ktab_t[:, 4:5], OP.is_equal, OP.mult)
                nc.vector.tensor_add(m1[:], m1[:], m2[:])
                w2_t = work.tile([P, FT], f32, tag="w2")
                nc.vector.tensor_add(w2_t[:], m0[:], m1[:])

                # y = z * w2, then downcast to bf16
                y_t = work.tile([P, FT], f32, tag="y")
                nc.vector.tensor_mul(y_t[:], z_t[:], w2_t[:])
                yb_t = work.tile([P, FT], bf16, tag="yb")
                nc.vector.tensor_copy(yb_t[:], y_t[:])
                ib_t = work.tile([P, FT], bf16, tag="ib")
                nc.vector.tensor_copy(ib_t[:], ig_t[:])

                # 16 masked reductions: acc[t*16+k] = sum((igbp==k)*y)
                sc_d = work.tile([P, FT], bf16, tag="scd")
                for k in range(N_IGBP):
                    nc.vector.scalar_tensor_tensor(
                        sc_d[:], ib_t[:], float(k), yb_t[:],
                        OP.is_equal, OP.mult,
                        accum_out=acc_t[:, t * N_IGBP + k: t * N_IGBP + k + 1],
                    )

                # balance: e = p0 - p1 + p2 (strided views, GPSIMD), e^2 on ScalarE
                p3 = p_t[:].rearrange("p (f c) -> p f c", c=C)
                e_t = work.tile([P, FT], f32, tag="e")
                nc.gpsimd.tensor_sub(e_t[:], p3[:, :, 0], p3[:, :, 1])
                nc.gpsimd.tensor_add(e_t[:], e_t[:], p3[:, :, 2])
                e2_t = work.tile([P, FT], f32, tag="e2")
                nc.scalar.activation(e2_t[:], e_t[:], AF.Square,
                                     accum_out=bal_t[:, t: t + 1])

            nc.sync.dma_start(acc_out[:], acc_t[:])
            nc.sync.dma_start(bal_out[:], bal_t[:])

    nc.finalize()
    return nc


def _run_spmd(in_maps, trace=False, trace_kwargs=None):
    if "nc" not in _CACHE:
        _CACHE["nc"] = _build()
    return run_bass_kernel_spmd(_CACHE["nc"], in_maps, list(range(N_CORES)),
                                trace=trace, **(trace_kwargs or {}))


def make_in_maps(preds, targets, nee_qc, igbp, koppen, koppen_table):
    preds = np.ascontiguousarray(preds, dtype=np.float32)
    targets = np.ascontiguousarray(targets, dtype=np.float32)
    nee_qc = np.ascontiguousarray(nee_qc, dtype=np.float32)
    igbp = np.ascontiguousarray(igbp, dtype=np.int32)
    koppen = np.ascontiguousarray(koppen, dtype=np.int32)

    ktab_np = np.tile(np.asarray(koppen_table, np.float32)[None, :], (P, 1))

    in_maps = []
    for m in range(N_CORES):
        b0, b1 = m * B_CORE, (m + 1) * B_CORE
        in_maps.append({
            "preds": preds[b0:b1].reshape(P, FP * C),
            "targs": targets[b0:b1].reshape(P, FP * C),
            "qc": nee_qc[b0:b1].reshape(P, FP),
            "igbp": igbp[b0:b1].reshape(P, FP),
            "kopp": koppen[b0:b1].reshape(P, FP),
            "ktab": ktab_np,
        })
    return in_maps


def finish(res, igbp_table):
    t1 = np.asarray(igbp_table, np.float64)
    mse_sum = 0.0
    bal_sum = 0.0
    for m in range(N_CORES):
        acc = res.results[m]["acc"].astype(np.float64)   # [P, NTILES*16]
        bal = res.results[m]["bal"].astype(np.float64)   # [P, NTILES]
        r_k = acc.reshape(P, NTILES, N_IGBP).sum(axis=(0, 1))   # [16]
        mse_sum += float((r_k * t1).sum())
        bal_sum += float(bal.sum())

    total = (mse_sum / C + ALPHA * bal_sum) / (B * T)
    return np.float32(total)


def kernel(preds, targets, nee_qc, igbp, koppen, igbp_table, koppen_table):
    in_maps = make_in_maps(preds, targets, nee_qc, igbp, koppen, koppen_table)
    res = _run_spmd(in_maps)
    return finish(res, igbp_table)


# revision 25
# speedup vs baseline: 1.0759x; 1.0759x over previous
"""Trainium2 Bass kernel for nn_CustomLoss_54400055771232.

Computes, over full inputs:
    mse   = mean_c (preds - targets)^2                      # [B, T]
    w     = nee_qc * igbp_table[igbp] * koppen_table[koppen]
    bal   = (preds[..2] + preds[..0] - preds[..1])^2        # [B, T]
    out   = mean_bt(mse * w + ALPHA * bal)                  # scalar

Strategy: pure data-parallel over B across 8 NeuronCores; per core the
[B/8 * T] domain is partition-chunked (partition p owns a contiguous run
of 5840 bt indices). Inputs are narrowed host-side (preds/targets/nee_qc
to bf16, index tensors to uint8), which halves HBM traffic and unlocks
the DVE 2x / ACT 4x perf modes; the resulting ~1e-5 relative error is
statistical (random rounding over 6M elements) and far below tolerance.

Per tile of the [B,T,C] stream: d = p - t (DVE bf16 2x), d^2 (ScalarE 4x,
in place), sum over C (GPSIMD strided adds), plus the balance term
(GPSIMD strided sub/add + ScalarE square with fused free-dim accum).
Then one untiled [B,T] stage: w2 = koppen_table[koppen] via exact
one-hot hats relu(T2[l]*(1-|kp-l|)) on ScalarE (table values as
per-partition scalar APs, so no recompile per call), y = s*q*w2, and the
igbp gather as 16 single-pass masked reductions on DVE
(scalar_tensor_tensor (ig==k)*y with fused accum_out). Host combines the
per-class sums with igbp_table in f64; the final mean is exact linear
post-processing. GPSIMD load is kept light because it shares an SBUF
port with the DVE (heavy GPSIMD use slows DVE ~3x).
"""

import sys

if "/opt/trn_rl_repo" not in sys.path:
    sys.path.insert(0, "/opt/trn_rl_repo")

import numpy as np
import ml_dtypes

import concourse.bass as bass
import concourse.bacc as bacc
import concourse.tile as tile
from concourse import mybir
from concourse.bass_utils import run_bass_kernel_spmd

# Problem constants (hardcoded per harness contract).
B, T, C = 16384, 365, 6
N_IGBP, N_KOPPEN = 16, 5
ALPHA = 0.1
N_CORES = 8

B_CORE = B // N_CORES            # 2048
BT = B_CORE * T                  # 747520
P = 128                          # partitions
FP = BT // P                     # 5840 free elems per partition (16*365)
FT = 730                         # bt elems per BTC-stage tile per partition
NTILES = FP // FT                # 8
assert FT * NTILES == FP

f32 = mybir.dt.float32
bf16 = mybir.dt.bfloat16
u8 = mybir.dt.uint8

AF = mybir.ActivationFunctionType
OP = mybir.AluOpType

_CACHE = {}


def _build():
    nc = bacc.Bacc("TRN2", target_bir_lowering=False, debug=False,
                   num_devices=N_CORES)

    preds = nc.dram_tensor("preds", [P, FP * C], bf16, kind="ExternalInput").ap()
    targs = nc.dram_tensor("targs", [P, FP * C], bf16, kind="ExternalInput").ap()
    qc = nc.dram_tensor("qc", [P, FP], bf16, kind="ExternalInput").ap()
    igbp = nc.dram_tensor("igbp", [P, FP], u8, kind="ExternalInput").ap()
    kopp = nc.dram_tensor("kopp", [P, FP], u8, kind="ExternalInput").ap()
    # coefficient columns, replicated per partition:
    #   0..4 koppen_table, 5..9 -l, 10..14 -koppen_table
    coef = nc.dram_tensor("coef", [P, 3 * N_KOPPEN], f32,
                          kind="ExternalInput").ap()
    # outputs: per-igbp-class mse partial sums (unweighted) + per-tile
    # balance sums
    acc_out = nc.dram_tensor("acc", [P, N_IGBP], f32, kind="ExternalOutput").ap()
    bal_out = nc.dram_tensor("bal", [P, NTILES], f32, kind="ExternalOutput").ap()

    preds3 = preds.rearrange("p (t f) -> p t f", t=NTILES)   # f = FT*C
    targs3 = targs.rearrange("p (t f) -> p t f", t=NTILES)

    with tile.TileContext(nc) as tc:
        with (
            tc.tile_pool(name="big", bufs=2) as big,     # BTC-stage tiles
            tc.tile_pool(name="work", bufs=2) as work,   # BTC-stage scratch
            tc.tile_pool(name="bt", bufs=1) as bt,       # [B,T]-stage tensors
            tc.tile_pool(name="accs", bufs=1) as accs,   # persistent
        ):
            coef_t = accs.tile([P, 3 * N_KOPPEN], f32)
            nc.sync.dma_start(coef_t[:], coef[:])
            t2ap = lambda l: coef_t[:, l: l + 1]
            negl = lambda l: coef_t[:, N_KOPPEN + l: N_KOPPEN + l + 1]
            negt2 = lambda l: coef_t[:, 2 * N_KOPPEN + l: 2 * N_KOPPEN + l + 1]
            acc_t = accs.tile([P, N_IGBP], f32)
            bal_t = accs.tile([P, NTILES], f32)

            s_full = bt.tile([P, FP], bf16)

            for t in range(NTILES):
                p_t = big.tile([P, FT * C], bf16, tag="p")
                nc.sync.dma_start(p_t[:], preds3[:, t, :])
                g_t = big.tile([P, FT * C], bf16, tag="tg")
                nc.sync.dma_start(g_t[:], targs3[:, t, :])

                # balance (GPSIMD strided + ScalarE square-accum)
                p3 = p_t[:].rearrange("p (f c) -> p f c", c=C)
                e_t = work.tile([P, FT], bf16, tag="e")
                nc.gpsimd.tensor_sub(e_t[:], p3[:, :, 0], p3[:, :, 1])
                nc.gpsimd.tensor_add(e_t[:], e_t[:], p3[:, :, 2])
                e2_t = work.tile([P, FT], bf16, tag="e2")
                nc.scalar.activation(e2_t[:], e_t[:], AF.Square,
                                     accum_out=bal_t[:, t: t + 1])

                # d = p - t in place into the targets tile (DVE bf16 2x),
                # then square in place (ScalarE 4x)
                nc.vector.tensor_sub(g_t[:], p_t[:], g_t[:])
                nc.scalar.activation(g_t[:], g_t[:], AF.Square)

                # s = sum over C=6 (GPSIMD strided adds) into s_full chunk
                g3 = g_t[:].rearrange("p (f c) -> p f c", c=C)
                r_t = work.tile([P, FT, 3], bf16, tag="r")
                nc.gpsimd.tensor_add(r_t[:], g3[:, :, 0:3], g3[:, :, 3:6])
                sv = s_full[:, t * FT: (t + 1) * FT]
                nc.gpsimd.tensor_add(sv[:], r_t[:, :, 0], r_t[:, :, 1])
                nc.gpsimd.tensor_add(sv[:], sv[:], r_t[:, :, 2])

            # ---- [B,T] stage, untiled ----
            q_t = bt.tile([P, FP], bf16)
            nc.sync.dma_start(q_t[:], qc[:])
            ig_t = bt.tile([P, FP], u8)
            nc.sync.dma_start(ig_t[:], igbp[:])
            kp_t = bt.tile([P, FP], u8)
            nc.sync.dma_start(kp_t[:], kopp[:])

            # z = s * q (DVE bf16 2x)
            z_t = bt.tile([P, FP], bf16)
            nc.vector.tensor_mul(z_t[:], s_full[:], q_t[:])

            # w2 = koppen_table[koppen] via exact one-hot hats on ScalarE:
            # relu(T2[l] * (1 - |kp - l|)); f32 outputs — bf16 here would
            # round each table value identically across its whole class
            # (systematic ~1e-3 bias, no cancellation). |kp-l| is an exact
            # small int, so the abs scratch stays bf16.
            w2_t = bt.tile([P, FP], f32)
            h0_t = bt.tile([P, FP], f32)
            h1_t = bt.tile([P, FP], f32)
            a_t = bt.tile([P, FP], bf16)
            add_eng = [None, None, nc.gpsimd, nc.gpsimd, nc.vector]
            for l in range(N_KOPPEN):
                nc.scalar.activation(a_t[:], kp_t[:], AF.Abs, bias=negl(l))
                m_t = w2_t if l == 0 else (h0_t if l == 1 else h1_t)
                nc.scalar.activation(m_t[:], a_t[:], AF.Relu,
                                     bias=t2ap(l), scale=negt2(l))
                if l >= 2:
                    add_eng[l].tensor_add(h0_t[:], h0_t[:], h1_t[:])
            nc.vector.tensor_add(w2_t[:], w2_t[:], h0_t[:])

            # y = z * w2 (DVE bf16 2x)
            y_t = bt.tile([P, FP], bf16)
            nc.vector.tensor_mul(y_t[:], z_t[:], w2_t[:])

            # igbp gather: 16 single-pass masked reductions on DVE
            sc_t = bt.tile([P, FP], bf16)
            for k in range(N_IGBP):
                nc.vector.scalar_tensor_tensor(
                    sc_t[:], ig_t[:], float(k), y_t[:],
                    OP.is_equal, OP.mult,
                    accum_out=acc_t[:, k: k + 1],
                )

            nc.sync.dma_start(acc_out[:], acc_t[:])
            nc.sync.dma_start(bal_out[:], bal_t[:])

    nc.finalize()
    return nc


def _run_spmd(in_maps, trace=False, trace_kwargs=None):
    if "nc" not in _CACHE:
        _CACHE["nc"] = _build()
    return run_bass_kernel_spmd(_CACHE["nc"], in_maps, list(range(N_CORES)),
                                trace=trace, **(trace_kwargs or {}))


def make_in_maps(preds, targets, nee_qc, igbp, koppen, igbp_table, koppen_table):
    bf = ml_dtypes.bfloat16
    preds = np.asarray(preds, np.float32).astype(bf)
    targets = np.asarray(targets, np.float32).astype(bf)
    nee_qc = np.asarray(nee_qc, np.float32).astype(bf)
    igbp = np.asarray(igbp).astype(np.uint8)
    koppen = np.asarray(koppen).astype(np.uint8)

    t2 = np.asarray(koppen_table, np.float32)
    coef_row = np.concatenate([t2, -np.arange(N_KOPPEN, dtype=np.float32), -t2])
    coef_np = np.tile(coef_row[None, :], (P, 1))

    in_maps = []
    for m in range(N_CORES):
        b0, b1 = m * B_CORE, (m + 1) * B_CORE
        in_maps.append({
            "preds": preds[b0:b1].reshape(P, FP * C),
            "targs": targets[b0:b1].reshape(P, FP * C),
            "qc": nee_qc[b0:b1].reshape(P, FP),
            "igbp": igbp[b0:b1].reshape(P, FP),
            "kopp": koppen[b0:b1].reshape(P, FP),
            "coef": coef_np,
        })
    return in_maps


def finish(res, igbp_table):
    t1 = np.asarray(igbp_table, np.float64)
    mse_sum = 0.0
    bal_sum = 0.0
    for m in range(N_CORES):
        acc = res.results[m]["acc"].astype(np.float64)   # [P, 16]
        bal = res.results[m]["bal"].astype(np.float64)   # [P, NTILES]
        mse_sum += float((acc.sum(axis=0) * t1).sum())
        bal_sum += float(bal.sum())
    total = (mse_sum / C + ALPHA * bal_sum) / (B * T)
    return np.float32(total)


def kernel(preds, targets, nee_qc, igbp, koppen, igbp_table, koppen_table):
    in_maps = make_in_maps(preds, targets, nee_qc, igbp, koppen,
                           igbp_table, koppen_table)
    res = _run_spmd(in_maps)
    return finish(res, igbp_table)


# revision 28
# speedup vs baseline: 1.1475x; 1.0665x over previous
"""Trainium2 Bass kernel for nn_CustomLoss_54400055771232.

Computes, over full inputs:
    mse   = mean_c (preds - targets)^2                      # [B, T]
    w     = nee_qc * igbp_table[igbp] * koppen_table[koppen]
    bal   = (preds[..2] + preds[..0] - preds[..1])^2        # [B, T]
    out   = mean_bt(mse * w + ALPHA * bal)                  # scalar

Strategy: pure data-parallel over B across 8 NeuronCores; per core the
[B/8 * T] domain is partition-chunked (partition p owns a contiguous run
of 5840 bt indices). Inputs are narrowed host-side (preds/targets/nee_qc
to bf16, index tensors to uint8), which halves HBM traffic and unlocks
the DVE 2x / ACT 4x perf modes; the resulting ~1e-5 relative error is
statistical (random rounding over 6M elements) and far below tolerance.

Per tile of the [B,T,C] stream: d = p - t (DVE bf16 2x), d^2 (ScalarE 4x,
in place), sum over C (GPSIMD strided adds), plus the balance term
(GPSIMD strided sub/add + ScalarE square with fused free-dim accum).
Then one untiled [B,T] stage: w2 = koppen_table[koppen] via exact
one-hot hats relu(T2[l]*(1-|kp-l|)) on ScalarE (table values as
per-partition scalar APs, so no recompile per call), y = s*q*w2, and the
igbp gather as 16 single-pass masked reductions on DVE
(scalar_tensor_tensor (ig==k)*y with fused accum_out). Host combines the
per-class sums with igbp_table in f64; the final mean is exact linear
post-processing. GPSIMD load is kept light because it shares an SBUF
port with the DVE (heavy GPSIMD use slows DVE ~3x).
"""

import sys

if "/opt/trn_rl_repo" not in sys.path:
    sys.path.insert(0, "/opt/trn_rl_repo")

import numpy as np
import ml_dtypes

import concourse.bass as bass
import concourse.bacc as bacc
import concourse.tile as tile
from concourse import mybir
from concourse.bass_utils import run_bass_kernel_spmd

# Problem constants (hardcoded per harness contract).
B, T, C = 16384, 365, 6
N_IGBP, N_KOPPEN = 16, 5
ALPHA = 0.1
N_CORES = 8

B_CORE = B // N_CORES            # 2048
BT = B_CORE * T                  # 747520
P = 128                          # partitions
FP = BT // P                     # 5840 free elems per partition (16*365)
FT = 730                         # bt elems per BTC-stage tile per partition
NTILES = FP // FT                # 8
assert FT * NTILES == FP

f32 = mybir.dt.float32
bf16 = mybir.dt.bfloat16
u8 = mybir.dt.uint8

AF = mybir.ActivationFunctionType
OP = mybir.AluOpType

_CACHE = {}


def _build():
    nc = bacc.Bacc("TRN2", target_bir_lowering=False, debug=False,
                   num_devices=N_CORES)

    preds = nc.dram_tensor("preds", [P, FP * C], bf16, kind="ExternalInput").ap()
    targs = nc.dram_tensor("targs", [P, FP * C], bf16, kind="ExternalInput").ap()
    qc = nc.dram_tensor("qc", [P, FP], bf16, kind="ExternalInput").ap()
    igbp = nc.dram_tensor("igbp", [P, FP], u8, kind="ExternalInput").ap()
    kopp = nc.dram_tensor("kopp", [P, FP], u8, kind="ExternalInput").ap()
    # coefficient columns, replicated per partition:
    #   0..4 koppen_table, 5..9 -l, 10..14 -koppen_table
    coef = nc.dram_tensor("coef", [P, 3 * N_KOPPEN], f32,
                          kind="ExternalInput").ap()
    # outputs: per-igbp-class mse partial sums (unweighted) + per-tile
    # balance sums
    acc_out = nc.dram_tensor("acc", [P, 2 * N_IGBP], f32, kind="ExternalOutput").ap()
    bal_out = nc.dram_tensor("bal", [P, NTILES], f32, kind="ExternalOutput").ap()

    preds3 = preds.rearrange("p (t f) -> p t f", t=NTILES)   # f = FT*C
    targs3 = targs.rearrange("p (t f) -> p t f", t=NTILES)

    with tile.TileContext(nc) as tc:
        with (
            tc.tile_pool(name="big", bufs=2) as big,     # BTC-stage tiles
            tc.tile_pool(name="work", bufs=2) as work,   # BTC-stage scratch
            tc.tile_pool(name="bt", bufs=1) as bt,       # [B,T]-stage tensors
            tc.tile_pool(name="accs", bufs=1) as accs,   # persistent
        ):
            coef_t = accs.tile([P, 3 * N_KOPPEN], f32)
            nc.sync.dma_start(coef_t[:], coef[:])
            t2ap = lambda l: coef_t[:, l: l + 1]
            negl = lambda l: coef_t[:, N_KOPPEN + l: N_KOPPEN + l + 1]
            negt2 = lambda l: coef_t[:, 2 * N_KOPPEN + l: 2 * N_KOPPEN + l + 1]
            acc_t = accs.tile([P, 2 * N_IGBP], f32)
            bal_t = accs.tile([P, NTILES], f32)

            s_full = bt.tile([P, FP], bf16)
            q_t = bt.tile([P, FP], bf16)
            nc.sync.dma_start(q_t[:], qc[:])
            ig_t = bt.tile([P, FP], u8)
            nc.sync.dma_start(ig_t[:], igbp[:])
            kp_t = bt.tile([P, FP], u8)
            nc.sync.dma_start(kp_t[:], kopp[:])

            # [B,T]-stage tensors, processed in halves so the second half of
            # the BTC stream overlaps the first half's gather pipeline
            NH = 2
            FH = FP // NH
            z_t = bt.tile([P, FP], bf16)
            w2_t = bt.tile([P, FP], f32)
            h0_t = bt.tile([P, FP], f32)
            h1_t = bt.tile([P, FP], f32)
            a_t = bt.tile([P, FP], bf16)
            y_t = bt.tile([P, FP], bf16)
            sc_t = bt.tile([P, FP], bf16)

            def bt_stage(h):
                sl = slice(h * FH, (h + 1) * FH)
                sf, qf, igf, kpf = s_full[:, sl], q_t[:, sl], ig_t[:, sl], kp_t[:, sl]
                z, w2, h0, h1, a, y, sc = (x[:, sl] for x in
                                           (z_t, w2_t, h0_t, h1_t, a_t, y_t, sc_t))
                nc.vector.tensor_mul(z[:], sf[:], qf[:])
                add_eng = [None, None, nc.gpsimd, nc.gpsimd, nc.vector]
                for l in range(N_KOPPEN):
                    nc.scalar.activation(a[:], kpf[:], AF.Abs, bias=negl(l))
                    m = w2 if l == 0 else (h0 if l == 1 else h1)
                    nc.scalar.activation(m[:], a[:], AF.Relu,
                                         bias=t2ap(l), scale=negt2(l))
                    if l >= 2:
                        add_eng[l].tensor_add(h0[:], h0[:], h1[:])
                nc.vector.tensor_add(w2[:], w2[:], h0[:])
                nc.vector.tensor_mul(y[:], z[:], w2[:])
                for k in range(N_IGBP):
                    nc.vector.scalar_tensor_tensor(
                        sc[:], igf[:], float(k), y[:],
                        OP.is_equal, OP.mult,
                        accum_out=acc_t[:, h * N_IGBP + k: h * N_IGBP + k + 1],
                    )

            for t in range(NTILES):
                p_t = big.tile([P, FT * C], bf16, tag="p")
                nc.sync.dma_start(p_t[:], preds3[:, t, :])
                g_t = big.tile([P, FT * C], bf16, tag="tg")
                nc.sync.dma_start(g_t[:], targs3[:, t, :])

                # balance (GPSIMD strided + ScalarE square-accum)
                p3 = p_t[:].rearrange("p (f c) -> p f c", c=C)
                e_t = work.tile([P, FT], bf16, tag="e")
                nc.gpsimd.tensor_sub(e_t[:], p3[:, :, 0], p3[:, :, 1])
                nc.gpsimd.tensor_add(e_t[:], e_t[:], p3[:, :, 2])
                e2_t = work.tile([P, FT], bf16, tag="e2")
                nc.scalar.activation(e2_t[:], e_t[:], AF.Square,
                                     accum_out=bal_t[:, t: t + 1])

                # d = p - t in place into the targets tile (DVE bf16 2x),
                # then square in place (ScalarE 4x)
                nc.vector.tensor_sub(g_t[:], p_t[:], g_t[:])
                nc.scalar.activation(g_t[:], g_t[:], AF.Square)

                # s = sum over C=6 (GPSIMD strided adds) into s_full chunk
                g3 = g_t[:].rearrange("p (f c) -> p f c", c=C)
                r_t = work.tile([P, FT, 3], bf16, tag="r")
                nc.gpsimd.tensor_add(r_t[:], g3[:, :, 0:3], g3[:, :, 3:6])
                sv = s_full[:, t * FT: (t + 1) * FT]
                nc.gpsimd.tensor_add(sv[:], r_t[:, :, 0], r_t[:, :, 1])
                nc.gpsimd.tensor_add(sv[:], sv[:], r_t[:, :, 2])

                if (t + 1) % (NTILES // NH) == 0:
                    bt_stage((t + 1) // (NTILES // NH) - 1)

            nc.sync.dma_start(acc_out[:], acc_t[:])
            nc.sync.dma_start(bal_out[:], bal_t[:])

    nc.finalize()
    return nc


def _run_spmd(in_maps, trace=False, trace_kwargs=None):
    if "nc" not in _CACHE:
        _CACHE["nc"] = _build()
    return run_bass_kernel_spmd(_CACHE["nc"], in_maps, list(range(N_CORES)),
                                trace=trace, **(trace_kwargs or {}))


def make_in_maps(preds, targets, nee_qc, igbp, koppen, igbp_table, koppen_table):
    bf = ml_dtypes.bfloat16
    preds = np.asarray(preds, np.float32).astype(bf)
    targets = np.asarray(targets, np.float32).astype(bf)
    nee_qc = np.asarray(nee_qc, np.float32).astype(bf)
    igbp = np.asarray(igbp).astype(np.uint8)
    koppen = np.asarray(koppen).astype(np.uint8)

    t2 = np.asarray(koppen_table, np.float32)
    coef_row = np.concatenate([t2, -np.arange(N_KOPPEN, dtype=np.float32), -t2])
    coef_np = np.tile(coef_row[None, :], (P, 1))

    in_maps = []
    for m in range(N_CORES):
        b0, b1 = m * B_CORE, (m + 1) * B_CORE
        in_maps.append({
            "preds": preds[b0:b1].reshape(P, FP * C),
            "targs": targets[b0:b1].reshape(P, FP * C),
            "qc": nee_qc[b0:b1].reshape(P, FP),
            "igbp": igbp[b0:b1].reshape(P, FP),
            "kopp": koppen[b0:b1].reshape(P, FP),
            "coef": coef_np,
        })
    return in_maps


def finish(res, igbp_table):
    t1 = np.asarray(igbp_table, np.float64)
    mse_sum = 0.0
    bal_sum = 0.0
    for m in range(N_CORES):
        acc = res.results[m]["acc"].astype(np.float64)   # [P, 2*16]
        bal = res.results[m]["bal"].astype(np.float64)   # [P, NTILES]
        r_k = acc.reshape(P, 2, N_IGBP).sum(axis=(0, 1))
        mse_sum += float((r_k * t1).sum())
        bal_sum += float(bal.sum())
    total = (mse_sum / C + ALPHA * bal_sum) / (B * T)
    return np.float32(total)


def kernel(preds, targets, nee_qc, igbp, koppen, igbp_table, koppen_table):
    in_maps = make_in_maps(preds, targets, nee_qc, igbp, koppen,
                           igbp_table, koppen_table)
    res = _run_spmd(in_maps)
    return finish(res, igbp_table)
